# revision 16
# baseline (speedup 1.0000x reference)
"""BlackholeEmbeddings Trainium2 kernel (8 NeuronCores, data-parallel).

Embedding lookup (word+pos+type) + sparse numeric-feature MLP + LayerNorm.
Sharding: sequence-parallel; core k owns positions [k*256,(k+1)*256) of all
8 batch rows (16 tiles of 128 positions per core, processed in 8 pairs).

The program is JIT-specialized on input structure (like weight folding):
 - any_active: whether any position has input_ids==NUM_TOKEN_ID with a
   non-NaN value (drives whether the numeric-MLP path is emitted at all;
   correctness holds for every input because kernel() inspects the actual
   inputs and compiles/selects the matching variant).
 - use_b2/use_g2/use_g1: non-default biases / norm affine params.

Text path: pos(+type, host-folded) prefill SBUF copy, then an indirect-DMA
gather of bf16 embedding rows with the SDMA CCE inline-add fused on top.
Tail: bn_stats/bn_aggr LayerNorm + ScalarE apply, bf16 output (host upcasts).
"""

import os
from contextlib import ExitStack

import ml_dtypes
import numpy as np

B, S, H, V = 8, 2048, 1024, 50257
NCORES = 8
SC = S // NCORES            # 256 positions per core
NT = B * (SC // 128)        # 16 tiles of 128 positions per core
NP = NT // 2                # 8 tile-pairs per core
NUM_TOKEN_ID = 5
NFEAT = 94
NF = 96                     # padded feature count (94 feats + ones + zero)
PI = 256                    # proj intermediate
C23 = 8388608.0             # 2**23
LN10INV = 0.43429448190325176
BF16 = ml_dtypes.bfloat16

_BUILD_CACHE = {}

TRACE = bool(int(os.environ.get("KBENCH_TRACE", "0")))
_LAST_RESULT = {}           # test.py reads exec_time_ns etc. from here


def _bcast_last(ap, n):
    """Append a broadcast (step-0) trailing axis of size n to an AP."""
    import concourse.bass as bass

    return bass.AP(tensor=ap.tensor, offset=ap.offset, ap=[*ap.ap, [0, n]])


def _build(any_active, use_b2, use_g2, use_g1):
    """Build + compile the (single, SPMD) Bass program."""
    import concourse.bass as bass
    import concourse.tile as tile
    from concourse import bacc, mybir
    from concourse.masks import make_identity

    dt = mybir.dt
    f32, bf, i32 = dt.float32, dt.bfloat16, dt.int32
    Alu = mybir.AluOpType
    Act = mybir.ActivationFunctionType

    nc = bacc.Bacc(
        "TRN2",
        target_bir_lowering=False,
        debug=False,
        enable_asserts=True,
        num_devices=NCORES,
    )

    ids_d = nc.dram_tensor("ids", [128, NT], i32, kind="ExternalInput")
    pos_d = nc.dram_tensor("pos", [128, 2, H], bf, kind="ExternalInput")
    wword_d = nc.dram_tensor("wword", [V, H], bf, kind="ExternalInput")
    if any_active:
        vals_d = nc.dram_tensor("vals", [128, NT], f32, kind="ExternalInput")
        fmt_d = nc.dram_tensor("fmt", [128, NT], i32, kind="ExternalInput")
        w1_d = nc.dram_tensor("w1", [NF, PI], bf, kind="ExternalInput")
        w2_d = nc.dram_tensor("w2", [PI, H], bf, kind="ExternalInput")
        if use_b2:
            b2_d = nc.dram_tensor("b2", [1, H], bf, kind="ExternalInput")
        if use_g2:
            g2_d = nc.dram_tensor("g2", [1, H], bf, kind="ExternalInput")
            bg2_d = nc.dram_tensor("bg2", [1, H], bf, kind="ExternalInput")
    if use_g1:
        g1_d = nc.dram_tensor("g1", [1, H], f32, kind="ExternalInput")
        bg1_d = nc.dram_tensor("bg1", [1, H], f32, kind="ExternalInput")
    out_d = nc.dram_tensor("out", [NT, 128, H], bf, kind="ExternalOutput")

    with tile.TileContext(nc) as tc, ExitStack() as ctx:
        const = ctx.enter_context(tc.tile_pool(name="const", bufs=1))
        gpool = ctx.enter_context(tc.tile_pool(name="gath", bufs=4))
        opool = ctx.enter_context(tc.tile_pool(name="oc", bufs=3))
        smpool = ctx.enter_context(tc.tile_pool(name="sm", bufs=4))
        if any_active:
            hpool = ctx.enter_context(tc.tile_pool(name="h", bufs=2))
            htpool = ctx.enter_context(tc.tile_pool(name="ht", bufs=4))
            tpool = ctx.enter_context(tc.tile_pool(name="tmp", bufs=2))
            ftspool = ctx.enter_context(tc.tile_pool(name="fts", bufs=2))
            pp_ft = ctx.enter_context(tc.tile_pool(name="ppx", bufs=2, space="PSUM"))
            pp_1 = ctx.enter_context(tc.tile_pool(name="pp1", bufs=1, space="PSUM"))
            pp_t = pp_ft
            pp_y = ctx.enter_context(tc.tile_pool(name="ppy", bufs=2, space="PSUM"))

        vec = nc.vector

        # ------------- inputs resident in SBUF (cheap ones first) -------------
        ids_sb = const.tile([128, NT], i32)
        nc.sync.dma_start(out=ids_sb[:], in_=ids_d.ap())
        pos01 = const.tile([128, 2, H], bf)
        nc.sync.dma_start(out=pos01[:], in_=pos_d.ap())
        eps12 = const.tile([128, 1], f32)
        vec.memset(eps12[:], 1e-12)
        if use_g1:
            g1_sb = const.tile([128, H], f32)
            nc.sync.dma_start(
                out=g1_sb[:],
                in_=bass.AP(tensor=g1_d, offset=0, ap=[[0, 128], [1, H]]),
            )
            bg1_sb = const.tile([128, H], f32)
            nc.sync.dma_start(
                out=bg1_sb[:],
                in_=bass.AP(tensor=bg1_d, offset=0, ap=[[0, 128], [1, H]]),
            )

        if any_active:
            vals_sb = const.tile([128, NT], f32)
            nc.sync.dma_start(out=vals_sb[:], in_=vals_d.ap())
            fmt_sb = const.tile([128, NT], i32)
            nc.sync.dma_start(out=fmt_sb[:], in_=fmt_d.ap())
            w1_sb = const.tile([NF, PI], bf)
            nc.sync.dma_start(out=w1_sb[:], in_=w1_d.ap())
            w2a_sb = const.tile([128, H], bf)
            nc.sync.dma_start(out=w2a_sb[:], in_=w2_d.ap()[0:128])
            w2b_sb = const.tile([128, H], bf)
            nc.sync.dma_start(out=w2b_sb[:], in_=w2_d.ap()[128:256])
            if use_b2:
                b2_sb = const.tile([1, H], bf)
                nc.sync.dma_start(out=b2_sb[:], in_=b2_d.ap())
                ones_row = const.tile([1, 128], bf)
                vec.memset(ones_row[:], 1.0)
            if use_g2:
                g2_sb = const.tile([128, H], bf)
                nc.sync.dma_start(
                    out=g2_sb[:],
                    in_=bass.AP(tensor=g2_d, offset=0, ap=[[0, 128], [1, H]]),
                )
                bg2_sb = const.tile([128, H], bf)
                nc.sync.dma_start(
                    out=bg2_sb[:],
                    in_=bass.AP(tensor=bg2_d, offset=0, ap=[[0, 128], [1, H]]),
                )

            ident = const.tile([128, 128], bf)
            make_identity(nc, ident[:])
            eps6 = const.tile([128, 1], f32)
            vec.memset(eps6[:], 1e-6)
            onesf = const.tile([128, NT], f32)
            vec.memset(onesf[:], 1.0)
            shamt23 = const.tile([128, NT, 23], i32)
            nc.gpsimd.iota(shamt23[:], pattern=[[0, NT], [1, 23]], base=0,
                           channel_multiplier=0)
            shamt11 = const.tile([128, NT, 11], i32)
            nc.gpsimd.iota(shamt11[:], pattern=[[0, NT], [1, 11]], base=0,
                           channel_multiplier=0)
            iota10f = const.tile([128, NT, 10], f32)
            nc.gpsimd.iota(
                iota10f[:], pattern=[[0, NT], [1, 10]], base=0, channel_multiplier=0,
                allow_small_or_imprecise_dtypes=True,
            )

            # ---------------- numeric features (all NT tiles at once) --------
            act_f = const.tile([128, NT], f32)
            act_i = const.tile([128, NT], i32)
            ti = const.tile([128, NT], i32)
            sv = const.tile([128, NT], f32)
            t1 = const.tile([128, NT], f32)
            t2 = const.tile([128, NT], f32)
            t3 = const.tile([128, NT], f32)
            av = const.tile([128, NT], f32)
            fl = const.tile([128, NT], f32)
            fl10 = const.tile([128, NT], f32)
            fl100 = const.tile([128, NT], f32)
            units = const.tile([128, NT], f32)
            tens = const.tile([128, NT], f32)
            m23 = const.tile([128, NT], i32)
            e8 = const.tile([128, NT], i32)
            e11 = const.tile([128, NT], i32)
            nz = const.tile([128, NT], i32)
            bsh = const.tile([128, NT, 23], i32)
            feats = const.tile([128, NT, NF], bf)

            # active = (ids == 5) & (vals == vals)
            vec.tensor_scalar(out=t1[:], in0=ids_sb[:], scalar1=float(NUM_TOKEN_ID),
                              scalar2=None, op0=Alu.is_equal)
            vec.tensor_tensor(out=t2[:], in0=vals_sb[:], in1=vals_sb[:],
                              op=Alu.is_equal)
            vec.tensor_tensor(out=act_f[:], in0=t1[:], in1=t2[:], op=Alu.mult)
            vec.tensor_copy(out=act_i[:], in_=act_f[:])
            # sv = active ? vals : 1.0 (copy-based select: NaN-safe)
            vec.select(out=sv[:], mask=act_i[:], on_true=vals_sb[:], on_false=onesf[:])

            bits = sv[:].bitcast(i32)
            vec.tensor_scalar(out=m23[:], in0=bits, scalar1=0x7FFFFF, scalar2=None,
                              op0=Alu.bitwise_and)
            vec.tensor_scalar(out=e8[:], in0=bits, scalar1=23, scalar2=0xFF,
                              op0=Alu.logical_shift_right, op1=Alu.bitwise_and)
            vec.memset(feats[:], 0.0)
            # double-precision mantissa bits: feats[29+j] = (m23 >> j) & 1
            vec.tensor_tensor(out=bsh[:], in0=_bcast_last(m23[:], 23), in1=shamt23[:],
                              op=Alu.logical_shift_right)
            vec.tensor_scalar(out=bsh[:], in0=bsh[:], scalar1=1, scalar2=None,
                              op0=Alu.bitwise_and)
            vec.tensor_copy(out=feats[:, :, 29:52], in_=bsh[:])
            # double exponent bits: e11 = (e8 + 896) * (e8 != 0)
            vec.tensor_scalar(out=e11[:], in0=e8[:], scalar1=896, scalar2=None,
                              op0=Alu.add)
            vec.tensor_scalar(out=nz[:], in0=e8[:], scalar1=0, scalar2=None,
                              op0=Alu.not_equal)
            vec.tensor_tensor(out=e11[:], in0=e11[:], in1=nz[:], op=Alu.mult)
            vec.tensor_tensor(out=bsh[:, :, 0:11], in0=_bcast_last(e11[:], 11),
                              in1=shamt11[:], op=Alu.logical_shift_right)
            vec.tensor_scalar(out=bsh[:, :, 0:11], in0=bsh[:, :, 0:11], scalar1=1,
                              scalar2=None, op0=Alu.bitwise_and)
            vec.tensor_copy(out=feats[:, :, 52:63], in_=bsh[:, :, 0:11])
            # av = |sv| via sign-bit clear
            vec.tensor_scalar(out=av[:].bitcast(i32), in0=bits, scalar1=0x7FFFFFFF,
                              scalar2=None, op0=Alu.bitwise_and)

            def floortrick(dst, src, guard_big=False):
                vec.tensor_scalar(out=t1[:], in0=src, scalar1=C23, scalar2=C23,
                                  op0=Alu.add, op1=Alu.subtract)
                vec.tensor_tensor(out=t2[:], in0=t1[:], in1=src, op=Alu.is_gt)
                vec.tensor_tensor(out=dst, in0=t1[:], in1=t2[:], op=Alu.subtract)
                if guard_big:
                    vec.tensor_scalar(out=ti[:], in0=src, scalar1=C23, scalar2=None,
                                      op0=Alu.is_ge)
                    vec.copy_predicated(out=dst, mask=ti[:], data=src)

            floortrick(fl[:], av[:], guard_big=True)
            vec.tensor_scalar(out=t3[:], in0=fl[:], scalar1=0.1, scalar2=None,
                              op0=Alu.mult)
            vec.tensor_copy(out=units[:], in_=t3[:])
            floortrick(fl10[:], units[:], guard_big=True)
            vec.tensor_scalar(out=t3[:], in0=fl10[:], scalar1=0.1, scalar2=None,
                              op0=Alu.mult)
            vec.tensor_copy(out=tens[:], in_=t3[:])
            floortrick(fl100[:], tens[:], guard_big=True)
            vec.tensor_scalar(out=t1[:], in0=fl10[:], scalar1=10.0, scalar2=None,
                              op0=Alu.mult)
            vec.tensor_tensor(out=units[:], in0=fl[:], in1=t1[:], op=Alu.subtract)
            vec.tensor_scalar(out=units[:], in0=units[:], scalar1=0.0, scalar2=9.0,
                              op0=Alu.max, op1=Alu.min)
            vec.tensor_scalar(out=t1[:], in0=fl100[:], scalar1=10.0, scalar2=None,
                              op0=Alu.mult)
            vec.tensor_tensor(out=tens[:], in0=fl10[:], in1=t1[:], op=Alu.subtract)
            vec.tensor_scalar(out=tens[:], in0=tens[:], scalar1=0.0, scalar2=9.0,
                              op0=Alu.max, op1=Alu.min)
            # one-hots
            vec.tensor_tensor(out=feats[:, :, 64:74], in0=_bcast_last(units[:], 10),
                              in1=iota10f[:], op=Alu.is_equal)
            vec.tensor_tensor(out=feats[:, :, 74:84], in0=_bcast_last(tens[:], 10),
                              in1=iota10f[:], op=Alu.is_equal)
            # ln(av) for large av via ln(1.m23) + (e8-127)*ln2 (Ln LUT range)
            lnbig = const.tile([128, NT], f32)
            mantf = const.tile([128, NT], i32)
            vec.tensor_scalar(out=mantf[:], in0=m23[:], scalar1=0x3F800000,
                              scalar2=None, op0=Alu.bitwise_or)
            nc.scalar.activation(out=lnbig[:], in_=mantf[:].bitcast(f32), func=Act.Ln,
                                 bias=0.0, scale=1.0)
            e8t = const.tile([128, NT], f32)
            vec.tensor_scalar(out=e8t[:], in0=e8[:], scalar1=127,
                              scalar2=0.6931471805599453,
                              op0=Alu.subtract, op1=Alu.mult)
            vec.tensor_tensor(out=lnbig[:], in0=lnbig[:], in1=e8t[:], op=Alu.add)
            smalls = const.tile([128, NT], i32)
            vec.tensor_scalar(out=smalls[:], in0=av[:], scalar1=1.0, scalar2=None,
                              op0=Alu.is_lt)
            # log_v = ln(av + 1e-6)
            vec.tensor_scalar(out=t3[:], in0=av[:], scalar1=1.0, scalar2=None,
                              op0=Alu.min)
            nc.scalar.activation(out=t3[:], in_=t3[:], func=Act.Ln, bias=eps6[:],
                                 scale=1.0)
            vec.tensor_copy(out=feats[:, :, 84], in_=lnbig[:])
            vec.copy_predicated(out=feats[:, :, 84], mask=smalls[:], data=t3[:])
            # sign
            vec.tensor_scalar(out=t1[:], in0=sv[:], scalar1=0.0, scalar2=None,
                              op0=Alu.is_gt)
            vec.tensor_scalar(out=t2[:], in0=sv[:], scalar1=0.0, scalar2=None,
                              op0=Alu.is_lt)
            vec.tensor_tensor(out=feats[:, :, 85], in0=t1[:], in1=t2[:],
                              op=Alu.subtract)
            # expo = floor(log10(max(av,eps))) * (av > 1e-6)
            vec.tensor_scalar(out=t3[:], in0=av[:], scalar1=1e-7, scalar2=1.0,
                              op0=Alu.max, op1=Alu.min)
            nc.scalar.activation(out=t3[:], in_=t3[:], func=Act.Ln, bias=0.0,
                                 scale=1.0)
            vec.copy_predicated(out=lnbig[:], mask=smalls[:], data=t3[:])
            vec.tensor_scalar(out=t3[:], in0=lnbig[:], scalar1=LN10INV, scalar2=None,
                              op0=Alu.mult)
            vec.tensor_scalar(out=t1[:], in0=t3[:], scalar1=C23, scalar2=C23,
                              op0=Alu.add, op1=Alu.subtract)
            vec.tensor_tensor(out=t2[:], in0=t1[:], in1=t3[:], op=Alu.is_gt)
            vec.tensor_tensor(out=t3[:], in0=t1[:], in1=t2[:], op=Alu.subtract)
            vec.tensor_scalar(out=t1[:], in0=av[:], scalar1=1e-6, scalar2=None,
                              op0=Alu.is_gt)
            vec.tensor_tensor(out=feats[:, :, 86], in0=t3[:], in1=t1[:], op=Alu.mult)
            # is_int / is_pos / is_zero / is_neg
            vec.tensor_tensor(out=feats[:, :, 87], in0=av[:], in1=fl[:],
                              op=Alu.is_equal)
            vec.tensor_scalar(out=feats[:, :, 88], in0=sv[:], scalar1=0.0,
                              scalar2=None, op0=Alu.is_gt)
            vec.tensor_scalar(out=feats[:, :, 89], in0=sv[:], scalar1=0.0,
                              scalar2=None, op0=Alu.is_equal)
            vec.tensor_scalar(out=feats[:, :, 90], in0=sv[:], scalar1=0.0,
                              scalar2=None, op0=Alu.is_lt)
            # is_pow2
            vec.tensor_scalar(out=t1[:], in0=m23[:], scalar1=0, scalar2=None,
                              op0=Alu.is_equal)
            vec.tensor_scalar(out=t2[:], in0=e8[:], scalar1=127, scalar2=None,
                              op0=Alu.is_ge)
            vec.tensor_tensor(out=t1[:], in0=t1[:], in1=t2[:], op=Alu.mult)
            vec.tensor_tensor(out=t2[:], in0=feats[:, :, 88], in1=feats[:, :, 87],
                              op=Alu.mult)
            vec.tensor_tensor(out=feats[:, :, 91], in0=t1[:], in1=t2[:], op=Alu.mult)
            # fmt one-hots
            vec.tensor_scalar(out=feats[:, :, 92], in0=fmt_sb[:], scalar1=0.0,
                              scalar2=None, op0=Alu.is_equal)
            vec.tensor_scalar(out=feats[:, :, 93], in0=fmt_sb[:], scalar1=1.0,
                              scalar2=None, op0=Alu.is_equal)
            vec.memset(feats[:, :, 94:95], 1.0)

        # ---------------- per-pair pipeline ----------------
        for P in range(NP):
            gth2 = gpool.tile([128, 2, H], bf, tag="gth")
            for t in range(2):
                nc.gpsimd.indirect_dma_start(
                    out=gth2[:, t, :],
                    out_offset=None,
                    in_=wword_d.ap(),
                    in_offset=bass.IndirectOffsetOnAxis(
                        ap=ids_sb[:, 2 * P + t : 2 * P + t + 1], axis=0),
                )
            # text = word + pos (in-place; alternate DVE / gpsimd accum-DMA
            # per pair to balance the two pacing engines)
            if P % 2 == 0:
                vec.tensor_tensor(out=gth2[:], in0=gth2[:], in1=pos01[:], op=Alu.add)
            else:
                nc.gpsimd.dma_start(out=gth2[:], in_=pos01[:], accum_op=Alu.add)

            if any_active:
                for t in range(2):
                    c = 2 * P + t
                    pft = pp_ft.tile([NF, 128], bf, tag="pt")
                    nc.tensor.transpose(out=pft[:], in_=feats[:, c, :],
                                        identity=ident[:])
                    fts = ftspool.tile([NF, 128], bf, tag="fts")
                    vec.tensor_copy(out=fts[:], in_=pft[:])
                    p1 = pp_1.tile([128, PI], f32, tag="p1")
                    nc.tensor.matmul(out=p1[:], lhsT=fts[:], rhs=w1_sb[:],
                                     start=True, stop=True)
                    h = hpool.tile([128, PI], bf, tag="h")
                    nc.scalar.activation(out=h[:], in_=p1[:], func=Act.Gelu,
                                         bias=0.0, scale=1.0)
                    pt0 = pp_t.tile([128, 128], bf, tag="pt")
                    nc.tensor.transpose(out=pt0[:], in_=h[:, 0:128],
                                        identity=ident[:])
                    ht0 = htpool.tile([128, 128], bf, tag="ht0")
                    vec.tensor_copy(out=ht0[:], in_=pt0[:])
                    pt1 = pp_t.tile([128, 128], bf, tag="pt")
                    nc.tensor.transpose(out=pt1[:], in_=h[:, 128:256],
                                        identity=ident[:])
                    ht1 = htpool.tile([128, 128], bf, tag="ht1")
                    vec.tensor_copy(out=ht1[:], in_=pt1[:])
                    py = pp_y.tile([128, H], f32, tag="py")
                    for nb in range(2):
                        sl = slice(nb * 512, (nb + 1) * 512)
                        nc.tensor.matmul(out=py[:, sl], lhsT=ht0[:],
                                         rhs=w2a_sb[:, sl], start=True, stop=False)
                        nc.tensor.matmul(out=py[:, sl], lhsT=ht1[:],
                                         rhs=w2b_sb[:, sl], start=False,
                                         stop=not use_b2)
                        if use_b2:
                            nc.tensor.matmul(out=py[:, sl], lhsT=ones_row[:],
                                             rhs=b2_sb[:, sl], start=False,
                                             stop=True)
                    st2 = smpool.tile([128, 2, 6], f32, tag="st2")
                    vec.bn_stats(out=st2[:, 0, :], in_=py[:, 0:512])
                    vec.bn_stats(out=st2[:, 1, :], in_=py[:, 512:1024])
                    mv2 = smpool.tile([128, 2], f32, tag="mv2")
                    vec.bn_aggr(out=mv2[:], in_=st2[:])
                    sd2 = smpool.tile([128, 1], f32, tag="sd2")
                    nc.scalar.activation(out=sd2[:], in_=mv2[:, 1:2], func=Act.Sqrt,
                                         bias=eps12[:], scale=1.0)
                    r2 = smpool.tile([128, 1], f32, tag="r2")
                    vec.reciprocal(out=r2[:], in_=sd2[:])
                    cm = smpool.tile([128, 1], f32, tag="cm")
                    vec.tensor_tensor(out=cm[:], in0=r2[:], in1=act_f[:, c : c + 1],
                                      op=Alu.mult)
                    dd = smpool.tile([128, 1], f32, tag="dd")
                    vec.tensor_scalar(out=dd[:], in0=mv2[:, 0:1], scalar1=cm[:],
                                      scalar2=-1.0, op0=Alu.mult, op1=Alu.mult)
                    tmp = tpool.tile([128, H], bf, tag="tmp")
                    nc.scalar.activation(out=tmp[:], in_=py[:], func=Act.Identity,
                                         bias=dd[:], scale=cm[:])
                    if use_g2:
                        vec.tensor_tensor(out=tmp[:], in0=tmp[:], in1=g2_sb[:],
                                          op=Alu.mult)
                        mb = tpool.tile([128, H], bf, tag="mb")
                        vec.tensor_scalar(out=mb[:], in0=bg2_sb[:],
                                          scalar1=act_f[:, c : c + 1],
                                          scalar2=None, op0=Alu.mult)
                        vec.tensor_tensor(out=tmp[:], in0=tmp[:], in1=mb[:],
                                          op=Alu.add)
                    vec.tensor_tensor(out=gth2[:, t, :], in0=gth2[:, t, :],
                                      in1=tmp[:], op=Alu.add)

            # ---- final LayerNorm on the pair ----
            stp = smpool.tile([128, 2, 2, 6], f32, tag="stp")
            for t in range(2):
                vec.bn_stats(out=stp[:, t, 0, :], in_=gth2[:, t, 0:512])
                vec.bn_stats(out=stp[:, t, 1, :], in_=gth2[:, t, 512:1024])
            mvp = smpool.tile([128, 2, 2], f32, tag="mvp")
            for t in range(2):
                vec.bn_aggr(out=mvp[:, t, :], in_=stp[:, t, :, :])
            sdp = smpool.tile([128, 2], f32, tag="sdp")
            nc.scalar.activation(out=sdp[:], in_=mvp[:, :, 1], func=Act.Sqrt,
                                 bias=eps12[:], scale=1.0)
            rp = smpool.tile([128, 2], f32, tag="rp")
            vec.reciprocal(out=rp[:], in_=sdp[:])
            nmrp = smpool.tile([128, 2], f32, tag="nmrp")
            nc.gpsimd.tensor_tensor(out=nmrp[:], in0=mvp[:, :, 0], in1=rp[:],
                                    op=Alu.mult)
            nc.gpsimd.tensor_scalar(out=nmrp[:], in0=nmrp[:], scalar1=-1.0,
                                    scalar2=None, op0=Alu.mult)

            oc2 = opool.tile([128, 2, H], bf, tag="oc")
            for t in range(2):
                nc.scalar.activation(out=oc2[:, t, :], in_=gth2[:, t, :],
                                     func=Act.Identity,
                                     bias=nmrp[:, t : t + 1], scale=rp[:, t : t + 1])
            if use_g1:
                vec.tensor_tensor(out=oc2[:], in0=oc2[:],
                                  in1=_bcast_mid(g1_sb[:]), op=Alu.mult)
                vec.tensor_tensor(out=oc2[:], in0=oc2[:],
                                  in1=_bcast_mid(bg1_sb[:]), op=Alu.add)

            out_ap = out_d.ap()[2 * P : 2 * P + 2].rearrange("c p h -> p c h")
            nc.sync.dma_start(out=out_ap, in_=oc2[:])

    nc.compile()
    return nc


def _bcast_mid(ap):
    """[128, H] -> [128, 2(broadcast), H]"""
    import concourse.bass as bass

    return bass.AP(tensor=ap.tensor, offset=ap.offset,
                   ap=[ap.ap[0], [0, 2], ap.ap[1]])


def _get_nc(flags):
    if flags not in _BUILD_CACHE:
        _BUILD_CACHE[flags] = _build(*flags)
    return _BUILD_CACHE[flags]


def _prep_maps(input_ids, numeric_values, numeric_formats, W_word, W_pos, W_type,
               ln_g, ln_b, p_w1, p_b1, p_w2, p_b2, pln_g, pln_b):
    ids32 = np.ascontiguousarray(input_ids.astype(np.int32))
    fmt32 = np.ascontiguousarray(numeric_formats.astype(np.int32))
    vals = np.ascontiguousarray(numeric_values.astype(np.float32))

    any_active = bool(((ids32 == NUM_TOKEN_ID) & ~np.isnan(vals)).any())
    wword = np.ascontiguousarray(W_word.astype(BF16))
    pos_prime = np.ascontiguousarray((W_pos[:S] + W_type[0]).astype(BF16))  # [S, H]

    w1a = np.zeros((NF, PI), np.float32)
    w1a[:NFEAT] = p_w1
    w1a[NFEAT] = p_b1
    w1a = np.ascontiguousarray(w1a.astype(BF16))
    w2 = np.ascontiguousarray(p_w2.astype(BF16))

    use_b2 = bool(np.any(p_b2 != 0))
    use_g2 = not (np.all(pln_g == 1.0) and np.all(pln_b == 0.0))
    use_g1 = not (np.all(ln_g == 1.0) and np.all(ln_b == 0.0))
    flags = (any_active, use_b2, use_g2, use_g1)

    in_maps = []
    for k in range(NCORES):
        sl = slice(k * SC, (k + 1) * SC)
        # [b, j, p] -> [p, b*2+j]
        ids_t = ids32[:, sl].reshape(B, 2, 128).transpose(2, 0, 1).reshape(128, NT)
        m = {
            "wword": wword,
            "pos": np.ascontiguousarray(
                pos_prime[sl].reshape(2, 128, H).transpose(1, 0, 2)),
            "ids": np.ascontiguousarray(ids_t),
        }
        if any_active:
            vals_t = vals[:, sl].reshape(B, 2, 128).transpose(2, 0, 1).reshape(128, NT)
            fmt_t = fmt32[:, sl].reshape(B, 2, 128).transpose(2, 0, 1).reshape(128, NT)
            m["vals"] = np.ascontiguousarray(vals_t)
            m["fmt"] = np.ascontiguousarray(fmt_t)
            m["w1"] = w1a
            m["w2"] = w2
            if use_b2:
                m["b2"] = np.ascontiguousarray(p_b2[None, :].astype(BF16))
            if use_g2:
                m["g2"] = np.ascontiguousarray(pln_g[None, :].astype(BF16))
                m["bg2"] = np.ascontiguousarray(pln_b[None, :].astype(BF16))
        if use_g1:
            m["g1"] = np.ascontiguousarray(ln_g[None, :].astype(np.float32))
            m["bg1"] = np.ascontiguousarray(ln_b[None, :].astype(np.float32))
        in_maps.append(m)
    return flags, in_maps


def _unshard(results):
    out = np.empty((B, S, H), np.float32)
    for k in range(NCORES):
        r = results[k]["out"].astype(np.float32)  # [NT, 128, H]
        out[:, k * SC : (k + 1) * SC, :] = r.reshape(B, 2, 128, H).reshape(B, SC, H)
    return out


def kernel(**inputs):
    from concourse.bass_utils import run_bass_kernel_spmd

    flags, in_maps = _prep_maps(**inputs)
    nc = _get_nc(flags)
    res = run_bass_kernel_spmd(
        nc, in_maps, core_ids=list(range(NCORES)), trace=TRACE,
    )
    _LAST_RESULT["exec_time_ns"] = res.exec_time_ns
    _LAST_RESULT["mean_exec_time_ns"] = res.mean_exec_time_ns
    _LAST_RESULT["trace"] = res.instructions_and_trace
    return _unshard(res.results)


# revision 17
# speedup vs baseline: 1.1110x; 1.1110x over previous
"""BlackholeEmbeddings Trainium2 kernel (8 NeuronCores, data-parallel).

Embedding lookup (word+pos+type) + sparse numeric-feature MLP + LayerNorm.
Sharding: sequence-parallel; core k owns positions [k*256,(k+1)*256) of all
8 batch rows (16 tiles of 128 positions per core, processed in 8 pairs).

The program is JIT-specialized on input structure (like weight folding):
 - any_active: whether any position has input_ids==NUM_TOKEN_ID with a
   non-NaN value (drives whether the numeric-MLP path is emitted at all;
   correctness holds for every input because kernel() inspects the actual
   inputs and compiles/selects the matching variant).
 - use_b2/use_g2/use_g1: non-default biases / norm affine params.

Text path: pos(+type, host-folded) prefill SBUF copy, then an indirect-DMA
gather of bf16 embedding rows with the SDMA CCE inline-add fused on top.
Tail: bn_stats/bn_aggr LayerNorm + ScalarE apply, bf16 output (host upcasts).
"""

import os
from contextlib import ExitStack

import ml_dtypes
import numpy as np

B, S, H, V = 8, 2048, 1024, 50257
NCORES = 8
SC = S // NCORES            # 256 positions per core
NT = B * (SC // 128)        # 16 tiles of 128 positions per core
NP = NT // 2                # 8 tile-pairs per core
NUM_TOKEN_ID = 5
NFEAT = 94
NF = 96                     # padded feature count (94 feats + ones + zero)
PI = 256                    # proj intermediate
C23 = 8388608.0             # 2**23
LN10INV = 0.43429448190325176
BF16 = ml_dtypes.bfloat16

_BUILD_CACHE = {}

TRACE = bool(int(os.environ.get("KBENCH_TRACE", "0")))
_LAST_RESULT = {}           # test.py reads exec_time_ns etc. from here


def _bcast_last(ap, n):
    """Append a broadcast (step-0) trailing axis of size n to an AP."""
    import concourse.bass as bass

    return bass.AP(tensor=ap.tensor, offset=ap.offset, ap=[*ap.ap, [0, n]])


def _build(any_active, use_b2, use_g2, use_g1):
    """Build + compile the (single, SPMD) Bass program."""
    import concourse.bass as bass
    import concourse.tile as tile
    from concourse import bacc, mybir
    from concourse.masks import make_identity

    dt = mybir.dt
    f32, bf, i32 = dt.float32, dt.bfloat16, dt.int32
    Alu = mybir.AluOpType
    Act = mybir.ActivationFunctionType

    nc = bacc.Bacc(
        "TRN2",
        target_bir_lowering=False,
        debug=False,
        enable_asserts=True,
        num_devices=NCORES,
    )

    ids_d = nc.dram_tensor("ids", [128, NT], i32, kind="ExternalInput")
    pos_d = nc.dram_tensor("pos", [128, 2, H], bf, kind="ExternalInput")
    wword_d = nc.dram_tensor("wword", [V, H], bf, kind="ExternalInput")
    if any_active:
        vals_d = nc.dram_tensor("vals", [128, NT], f32, kind="ExternalInput")
        fmt_d = nc.dram_tensor("fmt", [128, NT], i32, kind="ExternalInput")
        w1_d = nc.dram_tensor("w1", [NF, PI], bf, kind="ExternalInput")
        w2_d = nc.dram_tensor("w2", [PI, H], bf, kind="ExternalInput")
        if use_b2:
            b2_d = nc.dram_tensor("b2", [1, H], bf, kind="ExternalInput")
        if use_g2:
            g2_d = nc.dram_tensor("g2", [1, H], bf, kind="ExternalInput")
            bg2_d = nc.dram_tensor("bg2", [1, H], bf, kind="ExternalInput")
    if use_g1:
        g1_d = nc.dram_tensor("g1", [1, H], f32, kind="ExternalInput")
        bg1_d = nc.dram_tensor("bg1", [1, H], f32, kind="ExternalInput")
    out_d = nc.dram_tensor("out", [NT, 128, H], bf, kind="ExternalOutput")

    with tile.TileContext(nc) as tc, ExitStack() as ctx:
        const = ctx.enter_context(tc.tile_pool(name="const", bufs=1))
        gpool = ctx.enter_context(tc.tile_pool(name="gath", bufs=4))
        opool = ctx.enter_context(tc.tile_pool(name="oc", bufs=3))
        smpool = ctx.enter_context(tc.tile_pool(name="sm", bufs=4))
        if any_active:
            hpool = ctx.enter_context(tc.tile_pool(name="h", bufs=2))
            htpool = ctx.enter_context(tc.tile_pool(name="ht", bufs=4))
            tpool = ctx.enter_context(tc.tile_pool(name="tmp", bufs=2))
            ftspool = ctx.enter_context(tc.tile_pool(name="fts", bufs=2))
            pp_ft = ctx.enter_context(tc.tile_pool(name="ppx", bufs=2, space="PSUM"))
            pp_1 = ctx.enter_context(tc.tile_pool(name="pp1", bufs=1, space="PSUM"))
            pp_t = pp_ft
            pp_y = ctx.enter_context(tc.tile_pool(name="ppy", bufs=2, space="PSUM"))

        vec = nc.vector

        # ------------- inputs resident in SBUF (cheap ones first) -------------
        ids_sb = const.tile([128, NT], i32)
        nc.sync.dma_start(out=ids_sb[:], in_=ids_d.ap())
        pos01 = const.tile([128, 2, H], bf)
        nc.sync.dma_start(out=pos01[:], in_=pos_d.ap())
        eps12 = const.tile([128, 1], f32)
        vec.memset(eps12[:], 1e-12)
        if use_g1:
            g1_sb = const.tile([128, H], f32)
            nc.sync.dma_start(
                out=g1_sb[:],
                in_=bass.AP(tensor=g1_d, offset=0, ap=[[0, 128], [1, H]]),
            )
            bg1_sb = const.tile([128, H], f32)
            nc.sync.dma_start(
                out=bg1_sb[:],
                in_=bass.AP(tensor=bg1_d, offset=0, ap=[[0, 128], [1, H]]),
            )

        if any_active:
            vals_sb = const.tile([128, NT], f32)
            nc.sync.dma_start(out=vals_sb[:], in_=vals_d.ap())
            fmt_sb = const.tile([128, NT], i32)
            nc.sync.dma_start(out=fmt_sb[:], in_=fmt_d.ap())
            w1_sb = const.tile([NF, PI], bf)
            nc.sync.dma_start(out=w1_sb[:], in_=w1_d.ap())
            w2a_sb = const.tile([128, H], bf)
            nc.sync.dma_start(out=w2a_sb[:], in_=w2_d.ap()[0:128])
            w2b_sb = const.tile([128, H], bf)
            nc.sync.dma_start(out=w2b_sb[:], in_=w2_d.ap()[128:256])
            if use_b2:
                b2_sb = const.tile([1, H], bf)
                nc.sync.dma_start(out=b2_sb[:], in_=b2_d.ap())
                ones_row = const.tile([1, 128], bf)
                vec.memset(ones_row[:], 1.0)
            if use_g2:
                g2_sb = const.tile([128, H], bf)
                nc.sync.dma_start(
                    out=g2_sb[:],
                    in_=bass.AP(tensor=g2_d, offset=0, ap=[[0, 128], [1, H]]),
                )
                bg2_sb = const.tile([128, H], bf)
                nc.sync.dma_start(
                    out=bg2_sb[:],
                    in_=bass.AP(tensor=bg2_d, offset=0, ap=[[0, 128], [1, H]]),
                )

            ident = const.tile([128, 128], bf)
            make_identity(nc, ident[:])
            eps6 = const.tile([128, 1], f32)
            vec.memset(eps6[:], 1e-6)
            onesf = const.tile([128, NT], f32)
            vec.memset(onesf[:], 1.0)
            shamt23 = const.tile([128, NT, 23], i32)
            nc.gpsimd.iota(shamt23[:], pattern=[[0, NT], [1, 23]], base=0,
                           channel_multiplier=0)
            shamt11 = const.tile([128, NT, 11], i32)
            nc.gpsimd.iota(shamt11[:], pattern=[[0, NT], [1, 11]], base=0,
                           channel_multiplier=0)
            iota10f = const.tile([128, NT, 10], f32)
            nc.gpsimd.iota(
                iota10f[:], pattern=[[0, NT], [1, 10]], base=0, channel_multiplier=0,
                allow_small_or_imprecise_dtypes=True,
            )

            # ---------------- numeric features (all NT tiles at once) --------
            act_f = const.tile([128, NT], f32)
            act_i = const.tile([128, NT], i32)
            ti = const.tile([128, NT], i32)
            sv = const.tile([128, NT], f32)
            t1 = const.tile([128, NT], f32)
            t2 = const.tile([128, NT], f32)
            t3 = const.tile([128, NT], f32)
            av = const.tile([128, NT], f32)
            fl = const.tile([128, NT], f32)
            fl10 = const.tile([128, NT], f32)
            fl100 = const.tile([128, NT], f32)
            units = const.tile([128, NT], f32)
            tens = const.tile([128, NT], f32)
            m23 = const.tile([128, NT], i32)
            e8 = const.tile([128, NT], i32)
            e11 = const.tile([128, NT], i32)
            nz = const.tile([128, NT], i32)
            bsh = const.tile([128, NT, 23], i32)
            feats = const.tile([128, NT, NF], bf)

            # active = (ids == 5) & (vals == vals)
            vec.tensor_scalar(out=t1[:], in0=ids_sb[:], scalar1=float(NUM_TOKEN_ID),
                              scalar2=None, op0=Alu.is_equal)
            vec.tensor_tensor(out=t2[:], in0=vals_sb[:], in1=vals_sb[:],
                              op=Alu.is_equal)
            vec.tensor_tensor(out=act_f[:], in0=t1[:], in1=t2[:], op=Alu.mult)
            vec.tensor_copy(out=act_i[:], in_=act_f[:])
            # sv = active ? vals : 1.0 (copy-based select: NaN-safe)
            vec.select(out=sv[:], mask=act_i[:], on_true=vals_sb[:], on_false=onesf[:])

            bits = sv[:].bitcast(i32)
            vec.tensor_scalar(out=m23[:], in0=bits, scalar1=0x7FFFFF, scalar2=None,
                              op0=Alu.bitwise_and)
            vec.tensor_scalar(out=e8[:], in0=bits, scalar1=23, scalar2=0xFF,
                              op0=Alu.logical_shift_right, op1=Alu.bitwise_and)
            vec.memset(feats[:], 0.0)
            # double-precision mantissa bits: feats[29+j] = (m23 >> j) & 1
            vec.tensor_tensor(out=bsh[:], in0=_bcast_last(m23[:], 23), in1=shamt23[:],
                              op=Alu.logical_shift_right)
            vec.tensor_scalar(out=bsh[:], in0=bsh[:], scalar1=1, scalar2=None,
                              op0=Alu.bitwise_and)
            vec.tensor_copy(out=feats[:, :, 29:52], in_=bsh[:])
            # double exponent bits: e11 = (e8 + 896) * (e8 != 0)
            vec.tensor_scalar(out=e11[:], in0=e8[:], scalar1=896, scalar2=None,
                              op0=Alu.add)
            vec.tensor_scalar(out=nz[:], in0=e8[:], scalar1=0, scalar2=None,
                              op0=Alu.not_equal)
            vec.tensor_tensor(out=e11[:], in0=e11[:], in1=nz[:], op=Alu.mult)
            vec.tensor_tensor(out=bsh[:, :, 0:11], in0=_bcast_last(e11[:], 11),
                              in1=shamt11[:], op=Alu.logical_shift_right)
            vec.tensor_scalar(out=bsh[:, :, 0:11], in0=bsh[:, :, 0:11], scalar1=1,
                              scalar2=None, op0=Alu.bitwise_and)
            vec.tensor_copy(out=feats[:, :, 52:63], in_=bsh[:, :, 0:11])
            # av = |sv| via sign-bit clear
            vec.tensor_scalar(out=av[:].bitcast(i32), in0=bits, scalar1=0x7FFFFFFF,
                              scalar2=None, op0=Alu.bitwise_and)

            def floortrick(dst, src, guard_big=False):
                vec.tensor_scalar(out=t1[:], in0=src, scalar1=C23, scalar2=C23,
                                  op0=Alu.add, op1=Alu.subtract)
                vec.tensor_tensor(out=t2[:], in0=t1[:], in1=src, op=Alu.is_gt)
                vec.tensor_tensor(out=dst, in0=t1[:], in1=t2[:], op=Alu.subtract)
                if guard_big:
                    vec.tensor_scalar(out=ti[:], in0=src, scalar1=C23, scalar2=None,
                                      op0=Alu.is_ge)
                    vec.copy_predicated(out=dst, mask=ti[:], data=src)

            floortrick(fl[:], av[:], guard_big=True)
            vec.tensor_scalar(out=t3[:], in0=fl[:], scalar1=0.1, scalar2=None,
                              op0=Alu.mult)
            vec.tensor_copy(out=units[:], in_=t3[:])
            floortrick(fl10[:], units[:], guard_big=True)
            vec.tensor_scalar(out=t3[:], in0=fl10[:], scalar1=0.1, scalar2=None,
                              op0=Alu.mult)
            vec.tensor_copy(out=tens[:], in_=t3[:])
            floortrick(fl100[:], tens[:], guard_big=True)
            vec.tensor_scalar(out=t1[:], in0=fl10[:], scalar1=10.0, scalar2=None,
                              op0=Alu.mult)
            vec.tensor_tensor(out=units[:], in0=fl[:], in1=t1[:], op=Alu.subtract)
            vec.tensor_scalar(out=units[:], in0=units[:], scalar1=0.0, scalar2=9.0,
                              op0=Alu.max, op1=Alu.min)
            vec.tensor_scalar(out=t1[:], in0=fl100[:], scalar1=10.0, scalar2=None,
                              op0=Alu.mult)
            vec.tensor_tensor(out=tens[:], in0=fl10[:], in1=t1[:], op=Alu.subtract)
            vec.tensor_scalar(out=tens[:], in0=tens[:], scalar1=0.0, scalar2=9.0,
                              op0=Alu.max, op1=Alu.min)
            # one-hots
            vec.tensor_tensor(out=feats[:, :, 64:74], in0=_bcast_last(units[:], 10),
                              in1=iota10f[:], op=Alu.is_equal)
            vec.tensor_tensor(out=feats[:, :, 74:84], in0=_bcast_last(tens[:], 10),
                              in1=iota10f[:], op=Alu.is_equal)
            # ln(av) for large av via ln(1.m23) + (e8-127)*ln2 (Ln LUT range)
            lnbig = const.tile([128, NT], f32)
            mantf = const.tile([128, NT], i32)
            vec.tensor_scalar(out=mantf[:], in0=m23[:], scalar1=0x3F800000,
                              scalar2=None, op0=Alu.bitwise_or)
            nc.scalar.activation(out=lnbig[:], in_=mantf[:].bitcast(f32), func=Act.Ln,
                                 bias=0.0, scale=1.0)
            e8t = const.tile([128, NT], f32)
            vec.tensor_scalar(out=e8t[:], in0=e8[:], scalar1=127,
                              scalar2=0.6931471805599453,
                              op0=Alu.subtract, op1=Alu.mult)
            vec.tensor_tensor(out=lnbig[:], in0=lnbig[:], in1=e8t[:], op=Alu.add)
            smalls = const.tile([128, NT], i32)
            vec.tensor_scalar(out=smalls[:], in0=av[:], scalar1=1.0, scalar2=None,
                              op0=Alu.is_lt)
            # log_v = ln(av + 1e-6)
            vec.tensor_scalar(out=t3[:], in0=av[:], scalar1=1.0, scalar2=None,
                              op0=Alu.min)
            nc.scalar.activation(out=t3[:], in_=t3[:], func=Act.Ln, bias=eps6[:],
                                 scale=1.0)
            vec.tensor_copy(out=feats[:, :, 84], in_=lnbig[:])
            vec.copy_predicated(out=feats[:, :, 84], mask=smalls[:], data=t3[:])
            # sign
            vec.tensor_scalar(out=t1[:], in0=sv[:], scalar1=0.0, scalar2=None,
                              op0=Alu.is_gt)
            vec.tensor_scalar(out=t2[:], in0=sv[:], scalar1=0.0, scalar2=None,
                              op0=Alu.is_lt)
            vec.tensor_tensor(out=feats[:, :, 85], in0=t1[:], in1=t2[:],
                              op=Alu.subtract)
            # expo = floor(log10(max(av,eps))) * (av > 1e-6)
            vec.tensor_scalar(out=t3[:], in0=av[:], scalar1=1e-7, scalar2=1.0,
                              op0=Alu.max, op1=Alu.min)
            nc.scalar.activation(out=t3[:], in_=t3[:], func=Act.Ln, bias=0.0,
                                 scale=1.0)
            vec.copy_predicated(out=lnbig[:], mask=smalls[:], data=t3[:])
            vec.tensor_scalar(out=t3[:], in0=lnbig[:], scalar1=LN10INV, scalar2=None,
                              op0=Alu.mult)
            vec.tensor_scalar(out=t1[:], in0=t3[:], scalar1=C23, scalar2=C23,
                              op0=Alu.add, op1=Alu.subtract)
            vec.tensor_tensor(out=t2[:], in0=t1[:], in1=t3[:], op=Alu.is_gt)
            vec.tensor_tensor(out=t3[:], in0=t1[:], in1=t2[:], op=Alu.subtract)
            vec.tensor_scalar(out=t1[:], in0=av[:], scalar1=1e-6, scalar2=None,
                              op0=Alu.is_gt)
            vec.tensor_tensor(out=feats[:, :, 86], in0=t3[:], in1=t1[:], op=Alu.mult)
            # is_int / is_pos / is_zero / is_neg
            vec.tensor_tensor(out=feats[:, :, 87], in0=av[:], in1=fl[:],
                              op=Alu.is_equal)
            vec.tensor_scalar(out=feats[:, :, 88], in0=sv[:], scalar1=0.0,
                              scalar2=None, op0=Alu.is_gt)
            vec.tensor_scalar(out=feats[:, :, 89], in0=sv[:], scalar1=0.0,
                              scalar2=None, op0=Alu.is_equal)
            vec.tensor_scalar(out=feats[:, :, 90], in0=sv[:], scalar1=0.0,
                              scalar2=None, op0=Alu.is_lt)
            # is_pow2
            vec.tensor_scalar(out=t1[:], in0=m23[:], scalar1=0, scalar2=None,
                              op0=Alu.is_equal)
            vec.tensor_scalar(out=t2[:], in0=e8[:], scalar1=127, scalar2=None,
                              op0=Alu.is_ge)
            vec.tensor_tensor(out=t1[:], in0=t1[:], in1=t2[:], op=Alu.mult)
            vec.tensor_tensor(out=t2[:], in0=feats[:, :, 88], in1=feats[:, :, 87],
                              op=Alu.mult)
            vec.tensor_tensor(out=feats[:, :, 91], in0=t1[:], in1=t2[:], op=Alu.mult)
            # fmt one-hots
            vec.tensor_scalar(out=feats[:, :, 92], in0=fmt_sb[:], scalar1=0.0,
                              scalar2=None, op0=Alu.is_equal)
            vec.tensor_scalar(out=feats[:, :, 93], in0=fmt_sb[:], scalar1=1.0,
                              scalar2=None, op0=Alu.is_equal)
            vec.memset(feats[:, :, 94:95], 1.0)

        # ---------------- per-pair pipeline ----------------
        for P in range(NP):
            gth2 = gpool.tile([128, 2, H], bf, tag="gth")
            for t in range(2):
                nc.gpsimd.indirect_dma_start(
                    out=gth2[:, t, :],
                    out_offset=None,
                    in_=wword_d.ap(),
                    in_offset=bass.IndirectOffsetOnAxis(
                        ap=ids_sb[:, 2 * P + t : 2 * P + t + 1], axis=0),
                )
            # text = word + pos (in-place, bf16 2x mode)
            vec.tensor_tensor(out=gth2[:], in0=gth2[:], in1=pos01[:], op=Alu.add)

            if any_active:
                for t in range(2):
                    c = 2 * P + t
                    pft = pp_ft.tile([NF, 128], bf, tag="pt")
                    nc.tensor.transpose(out=pft[:], in_=feats[:, c, :],
                                        identity=ident[:])
                    fts = ftspool.tile([NF, 128], bf, tag="fts")
                    vec.tensor_copy(out=fts[:], in_=pft[:])
                    p1 = pp_1.tile([128, PI], f32, tag="p1")
                    nc.tensor.matmul(out=p1[:], lhsT=fts[:], rhs=w1_sb[:],
                                     start=True, stop=True)
                    h = hpool.tile([128, PI], bf, tag="h")
                    nc.scalar.activation(out=h[:], in_=p1[:], func=Act.Gelu,
                                         bias=0.0, scale=1.0)
                    pt0 = pp_t.tile([128, 128], bf, tag="pt")
                    nc.tensor.transpose(out=pt0[:], in_=h[:, 0:128],
                                        identity=ident[:])
                    ht0 = htpool.tile([128, 128], bf, tag="ht0")
                    vec.tensor_copy(out=ht0[:], in_=pt0[:])
                    pt1 = pp_t.tile([128, 128], bf, tag="pt")
                    nc.tensor.transpose(out=pt1[:], in_=h[:, 128:256],
                                        identity=ident[:])
                    ht1 = htpool.tile([128, 128], bf, tag="ht1")
                    vec.tensor_copy(out=ht1[:], in_=pt1[:])
                    py = pp_y.tile([128, H], f32, tag="py")
                    for nb in range(2):
                        sl = slice(nb * 512, (nb + 1) * 512)
                        nc.tensor.matmul(out=py[:, sl], lhsT=ht0[:],
                                         rhs=w2a_sb[:, sl], start=True, stop=False)
                        nc.tensor.matmul(out=py[:, sl], lhsT=ht1[:],
                                         rhs=w2b_sb[:, sl], start=False,
                                         stop=not use_b2)
                        if use_b2:
                            nc.tensor.matmul(out=py[:, sl], lhsT=ones_row[:],
                                             rhs=b2_sb[:, sl], start=False,
                                             stop=True)
                    st2 = smpool.tile([128, 2, 6], f32, tag="st2")
                    vec.bn_stats(out=st2[:, 0, :], in_=py[:, 0:512])
                    vec.bn_stats(out=st2[:, 1, :], in_=py[:, 512:1024])
                    mv2 = smpool.tile([128, 2], f32, tag="mv2")
                    vec.bn_aggr(out=mv2[:], in_=st2[:])
                    sd2 = smpool.tile([128, 1], f32, tag="sd2")
                    nc.scalar.activation(out=sd2[:], in_=mv2[:, 1:2], func=Act.Sqrt,
                                         bias=eps12[:], scale=1.0)
                    r2 = smpool.tile([128, 1], f32, tag="r2")
                    vec.reciprocal(out=r2[:], in_=sd2[:])
                    cm = smpool.tile([128, 1], f32, tag="cm")
                    vec.tensor_tensor(out=cm[:], in0=r2[:], in1=act_f[:, c : c + 1],
                                      op=Alu.mult)
                    dd = smpool.tile([128, 1], f32, tag="dd")
                    vec.tensor_scalar(out=dd[:], in0=mv2[:, 0:1], scalar1=cm[:],
                                      scalar2=-1.0, op0=Alu.mult, op1=Alu.mult)
                    tmp = tpool.tile([128, H], bf, tag="tmp")
                    nc.scalar.activation(out=tmp[:], in_=py[:], func=Act.Identity,
                                         bias=dd[:], scale=cm[:])
                    if use_g2:
                        vec.tensor_tensor(out=tmp[:], in0=tmp[:], in1=g2_sb[:],
                                          op=Alu.mult)
                        mb = tpool.tile([128, H], bf, tag="mb")
                        vec.tensor_scalar(out=mb[:], in0=bg2_sb[:],
                                          scalar1=act_f[:, c : c + 1],
                                          scalar2=None, op0=Alu.mult)
                        vec.tensor_tensor(out=tmp[:], in0=tmp[:], in1=mb[:],
                                          op=Alu.add)
                    vec.tensor_tensor(out=gth2[:, t, :], in0=gth2[:, t, :],
                                      in1=tmp[:], op=Alu.add)

            # ---- final LayerNorm on the pair ----
            stp = smpool.tile([128, 2, 2, 6], f32, tag="stp")
            for t in range(2):
                vec.bn_stats(out=stp[:, t, 0, :], in_=gth2[:, t, 0:512])
                vec.bn_stats(out=stp[:, t, 1, :], in_=gth2[:, t, 512:1024])
            mvp = smpool.tile([128, 2, 2], f32, tag="mvp")
            for t in range(2):
                vec.bn_aggr(out=mvp[:, t, :], in_=stp[:, t, :, :])
            sdp = smpool.tile([128, 2], f32, tag="sdp")
            nc.scalar.activation(out=sdp[:], in_=mvp[:, :, 1], func=Act.Sqrt,
                                 bias=eps12[:], scale=1.0)
            rp = smpool.tile([128, 2], f32, tag="rp")
            vec.reciprocal(out=rp[:], in_=sdp[:])
            nmrp = smpool.tile([128, 2], f32, tag="nmrp")
            nc.gpsimd.tensor_tensor(out=nmrp[:], in0=mvp[:, :, 0], in1=rp[:],
                                    op=Alu.mult)
            nc.gpsimd.tensor_scalar(out=nmrp[:], in0=nmrp[:], scalar1=-1.0,
                                    scalar2=None, op0=Alu.mult)

            oc2 = opool.tile([128, 2, H], bf, tag="oc")
            for t in range(2):
                nc.scalar.activation(out=oc2[:, t, :], in_=gth2[:, t, :],
                                     func=Act.Identity,
                                     bias=nmrp[:, t : t + 1], scale=rp[:, t : t + 1])
            if use_g1:
                vec.tensor_tensor(out=oc2[:], in0=oc2[:],
                                  in1=_bcast_mid(g1_sb[:]), op=Alu.mult)
                vec.tensor_tensor(out=oc2[:], in0=oc2[:],
                                  in1=_bcast_mid(bg1_sb[:]), op=Alu.add)

            out_ap = out_d.ap()[2 * P : 2 * P + 2].rearrange("c p h -> p c h")
            nc.sync.dma_start(out=out_ap, in_=oc2[:])

    nc.compile()
    return nc


def _bcast_mid(ap):
    """[128, H] -> [128, 2(broadcast), H]"""
    import concourse.bass as bass

    return bass.AP(tensor=ap.tensor, offset=ap.offset,
                   ap=[ap.ap[0], [0, 2], ap.ap[1]])


def _get_nc(flags):
    if flags not in _BUILD_CACHE:
        _BUILD_CACHE[flags] = _build(*flags)
    return _BUILD_CACHE[flags]


def _prep_maps(input_ids, numeric_values, numeric_formats, W_word, W_pos, W_type,
               ln_g, ln_b, p_w1, p_b1, p_w2, p_b2, pln_g, pln_b):
    ids32 = np.ascontiguousarray(input_ids.astype(np.int32))
    fmt32 = np.ascontiguousarray(numeric_formats.astype(np.int32))
    vals = np.ascontiguousarray(numeric_values.astype(np.float32))

    any_active = bool(((ids32 == NUM_TOKEN_ID) & ~np.isnan(vals)).any())
    wword = np.ascontiguousarray(W_word.astype(BF16))
    pos_prime = np.ascontiguousarray((W_pos[:S] + W_type[0]).astype(BF16))  # [S, H]

    w1a = np.zeros((NF, PI), np.float32)
    w1a[:NFEAT] = p_w1
    w1a[NFEAT] = p_b1
    w1a = np.ascontiguousarray(w1a.astype(BF16))
    w2 = np.ascontiguousarray(p_w2.astype(BF16))

    use_b2 = bool(np.any(p_b2 != 0))
    use_g2 = not (np.all(pln_g == 1.0) and np.all(pln_b == 0.0))
    use_g1 = not (np.all(ln_g == 1.0) and np.all(ln_b == 0.0))
    flags = (any_active, use_b2, use_g2, use_g1)

    in_maps = []
    for k in range(NCORES):
        sl = slice(k * SC, (k + 1) * SC)
        # [b, j, p] -> [p, b*2+j]
        ids_t = ids32[:, sl].reshape(B, 2, 128).transpose(2, 0, 1).reshape(128, NT)
        m = {
            "wword": wword,
            "pos": np.ascontiguousarray(
                pos_prime[sl].reshape(2, 128, H).transpose(1, 0, 2)),
            "ids": np.ascontiguousarray(ids_t),
        }
        if any_active:
            vals_t = vals[:, sl].reshape(B, 2, 128).transpose(2, 0, 1).reshape(128, NT)
            fmt_t = fmt32[:, sl].reshape(B, 2, 128).transpose(2, 0, 1).reshape(128, NT)
            m["vals"] = np.ascontiguousarray(vals_t)
            m["fmt"] = np.ascontiguousarray(fmt_t)
            m["w1"] = w1a
            m["w2"] = w2
            if use_b2:
                m["b2"] = np.ascontiguousarray(p_b2[None, :].astype(BF16))
            if use_g2:
                m["g2"] = np.ascontiguousarray(pln_g[None, :].astype(BF16))
                m["bg2"] = np.ascontiguousarray(pln_b[None, :].astype(BF16))
        if use_g1:
            m["g1"] = np.ascontiguousarray(ln_g[None, :].astype(np.float32))
            m["bg1"] = np.ascontiguousarray(ln_b[None, :].astype(np.float32))
        in_maps.append(m)
    return flags, in_maps


def _unshard(results):
    out = np.empty((B, S, H), np.float32)
    for k in range(NCORES):
        r = results[k]["out"].astype(np.float32)  # [NT, 128, H]
        out[:, k * SC : (k + 1) * SC, :] = r.reshape(B, 2, 128, H).reshape(B, SC, H)
    return out


def kernel(**inputs):
    from concourse.bass_utils import run_bass_kernel_spmd

    flags, in_maps = _prep_maps(**inputs)
    nc = _get_nc(flags)
    res = run_bass_kernel_spmd(
        nc, in_maps, core_ids=list(range(NCORES)), trace=TRACE,
    )
    _LAST_RESULT["exec_time_ns"] = res.exec_time_ns
    _LAST_RESULT["mean_exec_time_ns"] = res.mean_exec_time_ns
    _LAST_RESULT["trace"] = res.instructions_and_trace
    return _unshard(res.results)


# revision 18
# speedup vs baseline: 1.2432x; 1.1189x over previous
"""BlackholeEmbeddings Trainium2 kernel (8 NeuronCores, data-parallel).

Embedding lookup (word+pos+type) + sparse numeric-feature MLP + LayerNorm.
Sharding: sequence-parallel; core k owns positions [k*256,(k+1)*256) of all
8 batch rows (16 tiles of 128 positions per core, processed in 8 pairs).

The program is JIT-specialized on input structure (like weight folding):
 - any_active: whether any position has input_ids==NUM_TOKEN_ID with a
   non-NaN value (drives whether the numeric-MLP path is emitted at all;
   correctness holds for every input because kernel() inspects the actual
   inputs and compiles/selects the matching variant).
 - use_b2/use_g2/use_g1: non-default biases / norm affine params.

Text path: pos(+type, host-folded) prefill SBUF copy, then an indirect-DMA
gather of bf16 embedding rows with the SDMA CCE inline-add fused on top.
Tail: bn_stats/bn_aggr LayerNorm + ScalarE apply, bf16 output (host upcasts).
"""

import os
from contextlib import ExitStack

import ml_dtypes
import numpy as np

B, S, H, V = 8, 2048, 1024, 50257
NCORES = 8
SC = S // NCORES            # 256 positions per core
NT = B * (SC // 128)        # 16 tiles of 128 positions per core
NP = NT // 2                # 8 tile-pairs per core
NUM_TOKEN_ID = 5
NFEAT = 94
NF = 96                     # padded feature count (94 feats + ones + zero)
PI = 256                    # proj intermediate
C23 = 8388608.0             # 2**23
LN10INV = 0.43429448190325176
BF16 = ml_dtypes.bfloat16

_BUILD_CACHE = {}

TRACE = bool(int(os.environ.get("KBENCH_TRACE", "0")))
_LAST_RESULT = {}           # test.py reads exec_time_ns etc. from here


def _bcast_last(ap, n):
    """Append a broadcast (step-0) trailing axis of size n to an AP."""
    import concourse.bass as bass

    return bass.AP(tensor=ap.tensor, offset=ap.offset, ap=[*ap.ap, [0, n]])


def _build(any_active, use_b2, use_g2, use_g1):
    """Build + compile the (single, SPMD) Bass program."""
    import concourse.bass as bass
    import concourse.tile as tile
    from concourse import bacc, mybir
    from concourse.masks import make_identity

    dt = mybir.dt
    f32, bf, i32 = dt.float32, dt.bfloat16, dt.int32
    Alu = mybir.AluOpType
    Act = mybir.ActivationFunctionType

    nc = bacc.Bacc(
        "TRN2",
        target_bir_lowering=False,
        debug=False,
        enable_asserts=True,
        num_devices=NCORES,
    )

    ids_d = nc.dram_tensor("ids", [128, NT], i32, kind="ExternalInput")
    pos_d = nc.dram_tensor("pos", [128, 2, H], bf, kind="ExternalInput")
    wword_d = nc.dram_tensor("wword", [V, H], bf, kind="ExternalInput")
    if any_active:
        vals_d = nc.dram_tensor("vals", [128, NT], f32, kind="ExternalInput")
        fmt_d = nc.dram_tensor("fmt", [128, NT], i32, kind="ExternalInput")
        w1_d = nc.dram_tensor("w1", [NF, PI], bf, kind="ExternalInput")
        w2_d = nc.dram_tensor("w2", [PI, H], bf, kind="ExternalInput")
        if use_b2:
            b2_d = nc.dram_tensor("b2", [1, H], bf, kind="ExternalInput")
        if use_g2:
            g2_d = nc.dram_tensor("g2", [1, H], bf, kind="ExternalInput")
            bg2_d = nc.dram_tensor("bg2", [1, H], bf, kind="ExternalInput")
    if use_g1:
        g1_d = nc.dram_tensor("g1", [1, H], f32, kind="ExternalInput")
        bg1_d = nc.dram_tensor("bg1", [1, H], f32, kind="ExternalInput")
    out_d = nc.dram_tensor("out", [NT, 128, H], bf, kind="ExternalOutput")

    with tile.TileContext(nc) as tc, ExitStack() as ctx:
        const = ctx.enter_context(tc.tile_pool(name="const", bufs=1))
        gpool = ctx.enter_context(tc.tile_pool(name="gath", bufs=4))
        opool = ctx.enter_context(tc.tile_pool(name="oc", bufs=3))
        smpool = ctx.enter_context(tc.tile_pool(name="sm", bufs=4))
        if any_active:
            hpool = ctx.enter_context(tc.tile_pool(name="h", bufs=2))
            htpool = ctx.enter_context(tc.tile_pool(name="ht", bufs=4))
            tpool = ctx.enter_context(tc.tile_pool(name="tmp", bufs=2))
            ftspool = ctx.enter_context(tc.tile_pool(name="fts", bufs=2))
            pp_ft = ctx.enter_context(tc.tile_pool(name="ppx", bufs=2, space="PSUM"))
            pp_1 = ctx.enter_context(tc.tile_pool(name="pp1", bufs=1, space="PSUM"))
            pp_t = pp_ft
            pp_y = ctx.enter_context(tc.tile_pool(name="ppy", bufs=2, space="PSUM"))

        vec = nc.vector

        # ------------- inputs resident in SBUF (cheap ones first) -------------
        ids_sb = const.tile([128, NT], i32)
        nc.sync.dma_start(out=ids_sb[:], in_=ids_d.ap())
        pos01 = const.tile([128, 2, H], bf)
        nc.sync.dma_start(out=pos01[:], in_=pos_d.ap())
        eps12 = const.tile([128, 1], f32)
        vec.memset(eps12[:], 1e-12)
        if use_g1:
            g1_sb = const.tile([128, H], f32)
            nc.sync.dma_start(
                out=g1_sb[:],
                in_=bass.AP(tensor=g1_d, offset=0, ap=[[0, 128], [1, H]]),
            )
            bg1_sb = const.tile([128, H], f32)
            nc.sync.dma_start(
                out=bg1_sb[:],
                in_=bass.AP(tensor=bg1_d, offset=0, ap=[[0, 128], [1, H]]),
            )

        if any_active:
            vals_sb = const.tile([128, NT], f32)
            nc.sync.dma_start(out=vals_sb[:], in_=vals_d.ap())
            fmt_sb = const.tile([128, NT], i32)
            nc.sync.dma_start(out=fmt_sb[:], in_=fmt_d.ap())
            w1_sb = const.tile([NF, PI], bf)
            nc.sync.dma_start(out=w1_sb[:], in_=w1_d.ap())
            w2a_sb = const.tile([128, H], bf)
            nc.sync.dma_start(out=w2a_sb[:], in_=w2_d.ap()[0:128])
            w2b_sb = const.tile([128, H], bf)
            nc.sync.dma_start(out=w2b_sb[:], in_=w2_d.ap()[128:256])
            if use_b2:
                b2_sb = const.tile([1, H], bf)
                nc.sync.dma_start(out=b2_sb[:], in_=b2_d.ap())
                ones_row = const.tile([1, 128], bf)
                vec.memset(ones_row[:], 1.0)
            if use_g2:
                g2_sb = const.tile([128, H], bf)
                nc.sync.dma_start(
                    out=g2_sb[:],
                    in_=bass.AP(tensor=g2_d, offset=0, ap=[[0, 128], [1, H]]),
                )
                bg2_sb = const.tile([128, H], bf)
                nc.sync.dma_start(
                    out=bg2_sb[:],
                    in_=bass.AP(tensor=bg2_d, offset=0, ap=[[0, 128], [1, H]]),
                )

            ident = const.tile([128, 128], bf)
            make_identity(nc, ident[:])
            eps6 = const.tile([128, 1], f32)
            vec.memset(eps6[:], 1e-6)
            onesf = const.tile([128, NT], f32)
            vec.memset(onesf[:], 1.0)
            shamt23 = const.tile([128, NT, 23], i32)
            nc.gpsimd.iota(shamt23[:], pattern=[[0, NT], [1, 23]], base=0,
                           channel_multiplier=0)
            shamt11 = const.tile([128, NT, 11], i32)
            nc.gpsimd.iota(shamt11[:], pattern=[[0, NT], [1, 11]], base=0,
                           channel_multiplier=0)
            iota10f = const.tile([128, NT, 10], f32)
            nc.gpsimd.iota(
                iota10f[:], pattern=[[0, NT], [1, 10]], base=0, channel_multiplier=0,
                allow_small_or_imprecise_dtypes=True,
            )

            # ---------------- numeric features (all NT tiles at once) --------
            act_f = const.tile([128, NT], f32)
            act_i = const.tile([128, NT], i32)
            ti = const.tile([128, NT], i32)
            sv = const.tile([128, NT], f32)
            t1 = const.tile([128, NT], f32)
            t2 = const.tile([128, NT], f32)
            t3 = const.tile([128, NT], f32)
            av = const.tile([128, NT], f32)
            fl = const.tile([128, NT], f32)
            fl10 = const.tile([128, NT], f32)
            fl100 = const.tile([128, NT], f32)
            units = const.tile([128, NT], f32)
            tens = const.tile([128, NT], f32)
            m23 = const.tile([128, NT], i32)
            e8 = const.tile([128, NT], i32)
            e11 = const.tile([128, NT], i32)
            nz = const.tile([128, NT], i32)
            bsh = const.tile([128, NT, 23], i32)
            feats = const.tile([128, NT, NF], bf)

            # active = (ids == 5) & (vals == vals)
            vec.tensor_scalar(out=t1[:], in0=ids_sb[:], scalar1=float(NUM_TOKEN_ID),
                              scalar2=None, op0=Alu.is_equal)
            vec.tensor_tensor(out=t2[:], in0=vals_sb[:], in1=vals_sb[:],
                              op=Alu.is_equal)
            vec.tensor_tensor(out=act_f[:], in0=t1[:], in1=t2[:], op=Alu.mult)
            vec.tensor_copy(out=act_i[:], in_=act_f[:])
            # sv = active ? vals : 1.0 (copy-based select: NaN-safe)
            vec.select(out=sv[:], mask=act_i[:], on_true=vals_sb[:], on_false=onesf[:])

            bits = sv[:].bitcast(i32)
            vec.tensor_scalar(out=m23[:], in0=bits, scalar1=0x7FFFFF, scalar2=None,
                              op0=Alu.bitwise_and)
            vec.tensor_scalar(out=e8[:], in0=bits, scalar1=23, scalar2=0xFF,
                              op0=Alu.logical_shift_right, op1=Alu.bitwise_and)
            vec.memset(feats[:], 0.0)
            # double-precision mantissa bits: feats[29+j] = (m23 >> j) & 1
            vec.tensor_tensor(out=bsh[:], in0=_bcast_last(m23[:], 23), in1=shamt23[:],
                              op=Alu.logical_shift_right)
            vec.tensor_scalar(out=bsh[:], in0=bsh[:], scalar1=1, scalar2=None,
                              op0=Alu.bitwise_and)
            vec.tensor_copy(out=feats[:, :, 29:52], in_=bsh[:])
            # double exponent bits: e11 = (e8 + 896) * (e8 != 0)
            vec.tensor_scalar(out=e11[:], in0=e8[:], scalar1=896, scalar2=None,
                              op0=Alu.add)
            vec.tensor_scalar(out=nz[:], in0=e8[:], scalar1=0, scalar2=None,
                              op0=Alu.not_equal)
            vec.tensor_tensor(out=e11[:], in0=e11[:], in1=nz[:], op=Alu.mult)
            vec.tensor_tensor(out=bsh[:, :, 0:11], in0=_bcast_last(e11[:], 11),
                              in1=shamt11[:], op=Alu.logical_shift_right)
            vec.tensor_scalar(out=bsh[:, :, 0:11], in0=bsh[:, :, 0:11], scalar1=1,
                              scalar2=None, op0=Alu.bitwise_and)
            vec.tensor_copy(out=feats[:, :, 52:63], in_=bsh[:, :, 0:11])
            # av = |sv| via sign-bit clear
            vec.tensor_scalar(out=av[:].bitcast(i32), in0=bits, scalar1=0x7FFFFFFF,
                              scalar2=None, op0=Alu.bitwise_and)

            def floortrick(dst, src, guard_big=False):
                vec.tensor_scalar(out=t1[:], in0=src, scalar1=C23, scalar2=C23,
                                  op0=Alu.add, op1=Alu.subtract)
                vec.tensor_tensor(out=t2[:], in0=t1[:], in1=src, op=Alu.is_gt)
                vec.tensor_tensor(out=dst, in0=t1[:], in1=t2[:], op=Alu.subtract)
                if guard_big:
                    vec.tensor_scalar(out=ti[:], in0=src, scalar1=C23, scalar2=None,
                                      op0=Alu.is_ge)
                    vec.copy_predicated(out=dst, mask=ti[:], data=src)

            floortrick(fl[:], av[:], guard_big=True)
            vec.tensor_scalar(out=t3[:], in0=fl[:], scalar1=0.1, scalar2=None,
                              op0=Alu.mult)
            vec.tensor_copy(out=units[:], in_=t3[:])
            floortrick(fl10[:], units[:], guard_big=True)
            vec.tensor_scalar(out=t3[:], in0=fl10[:], scalar1=0.1, scalar2=None,
                              op0=Alu.mult)
            vec.tensor_copy(out=tens[:], in_=t3[:])
            floortrick(fl100[:], tens[:], guard_big=True)
            vec.tensor_scalar(out=t1[:], in0=fl10[:], scalar1=10.0, scalar2=None,
                              op0=Alu.mult)
            vec.tensor_tensor(out=units[:], in0=fl[:], in1=t1[:], op=Alu.subtract)
            vec.tensor_scalar(out=units[:], in0=units[:], scalar1=0.0, scalar2=9.0,
                              op0=Alu.max, op1=Alu.min)
            vec.tensor_scalar(out=t1[:], in0=fl100[:], scalar1=10.0, scalar2=None,
                              op0=Alu.mult)
            vec.tensor_tensor(out=tens[:], in0=fl10[:], in1=t1[:], op=Alu.subtract)
            vec.tensor_scalar(out=tens[:], in0=tens[:], scalar1=0.0, scalar2=9.0,
                              op0=Alu.max, op1=Alu.min)
            # one-hots
            vec.tensor_tensor(out=feats[:, :, 64:74], in0=_bcast_last(units[:], 10),
                              in1=iota10f[:], op=Alu.is_equal)
            vec.tensor_tensor(out=feats[:, :, 74:84], in0=_bcast_last(tens[:], 10),
                              in1=iota10f[:], op=Alu.is_equal)
            # ln(av) for large av via ln(1.m23) + (e8-127)*ln2 (Ln LUT range)
            lnbig = const.tile([128, NT], f32)
            mantf = const.tile([128, NT], i32)
            vec.tensor_scalar(out=mantf[:], in0=m23[:], scalar1=0x3F800000,
                              scalar2=None, op0=Alu.bitwise_or)
            nc.scalar.activation(out=lnbig[:], in_=mantf[:].bitcast(f32), func=Act.Ln,
                                 bias=0.0, scale=1.0)
            e8t = const.tile([128, NT], f32)
            vec.tensor_scalar(out=e8t[:], in0=e8[:], scalar1=127,
                              scalar2=0.6931471805599453,
                              op0=Alu.subtract, op1=Alu.mult)
            vec.tensor_tensor(out=lnbig[:], in0=lnbig[:], in1=e8t[:], op=Alu.add)
            smalls = const.tile([128, NT], i32)
            vec.tensor_scalar(out=smalls[:], in0=av[:], scalar1=1.0, scalar2=None,
                              op0=Alu.is_lt)
            # log_v = ln(av + 1e-6)
            vec.tensor_scalar(out=t3[:], in0=av[:], scalar1=1.0, scalar2=None,
                              op0=Alu.min)
            nc.scalar.activation(out=t3[:], in_=t3[:], func=Act.Ln, bias=eps6[:],
                                 scale=1.0)
            vec.tensor_copy(out=feats[:, :, 84], in_=lnbig[:])
            vec.copy_predicated(out=feats[:, :, 84], mask=smalls[:], data=t3[:])
            # sign
            vec.tensor_scalar(out=t1[:], in0=sv[:], scalar1=0.0, scalar2=None,
                              op0=Alu.is_gt)
            vec.tensor_scalar(out=t2[:], in0=sv[:], scalar1=0.0, scalar2=None,
                              op0=Alu.is_lt)
            vec.tensor_tensor(out=feats[:, :, 85], in0=t1[:], in1=t2[:],
                              op=Alu.subtract)
            # expo = floor(log10(max(av,eps))) * (av > 1e-6)
            vec.tensor_scalar(out=t3[:], in0=av[:], scalar1=1e-7, scalar2=1.0,
                              op0=Alu.max, op1=Alu.min)
            nc.scalar.activation(out=t3[:], in_=t3[:], func=Act.Ln, bias=0.0,
                                 scale=1.0)
            vec.copy_predicated(out=lnbig[:], mask=smalls[:], data=t3[:])
            vec.tensor_scalar(out=t3[:], in0=lnbig[:], scalar1=LN10INV, scalar2=None,
                              op0=Alu.mult)
            vec.tensor_scalar(out=t1[:], in0=t3[:], scalar1=C23, scalar2=C23,
                              op0=Alu.add, op1=Alu.subtract)
            vec.tensor_tensor(out=t2[:], in0=t1[:], in1=t3[:], op=Alu.is_gt)
            vec.tensor_tensor(out=t3[:], in0=t1[:], in1=t2[:], op=Alu.subtract)
            vec.tensor_scalar(out=t1[:], in0=av[:], scalar1=1e-6, scalar2=None,
                              op0=Alu.is_gt)
            vec.tensor_tensor(out=feats[:, :, 86], in0=t3[:], in1=t1[:], op=Alu.mult)
            # is_int / is_pos / is_zero / is_neg
            vec.tensor_tensor(out=feats[:, :, 87], in0=av[:], in1=fl[:],
                              op=Alu.is_equal)
            vec.tensor_scalar(out=feats[:, :, 88], in0=sv[:], scalar1=0.0,
                              scalar2=None, op0=Alu.is_gt)
            vec.tensor_scalar(out=feats[:, :, 89], in0=sv[:], scalar1=0.0,
                              scalar2=None, op0=Alu.is_equal)
            vec.tensor_scalar(out=feats[:, :, 90], in0=sv[:], scalar1=0.0,
                              scalar2=None, op0=Alu.is_lt)
            # is_pow2
            vec.tensor_scalar(out=t1[:], in0=m23[:], scalar1=0, scalar2=None,
                              op0=Alu.is_equal)
            vec.tensor_scalar(out=t2[:], in0=e8[:], scalar1=127, scalar2=None,
                              op0=Alu.is_ge)
            vec.tensor_tensor(out=t1[:], in0=t1[:], in1=t2[:], op=Alu.mult)
            vec.tensor_tensor(out=t2[:], in0=feats[:, :, 88], in1=feats[:, :, 87],
                              op=Alu.mult)
            vec.tensor_tensor(out=feats[:, :, 91], in0=t1[:], in1=t2[:], op=Alu.mult)
            # fmt one-hots
            vec.tensor_scalar(out=feats[:, :, 92], in0=fmt_sb[:], scalar1=0.0,
                              scalar2=None, op0=Alu.is_equal)
            vec.tensor_scalar(out=feats[:, :, 93], in0=fmt_sb[:], scalar1=1.0,
                              scalar2=None, op0=Alu.is_equal)
            vec.memset(feats[:, :, 94:95], 1.0)

        # ---------------- per-pair pipeline ----------------
        for P in range(NP):
            gth2 = gpool.tile([128, 2, H], bf, tag="gth")
            for t in range(2):
                nc.gpsimd.indirect_dma_start(
                    out=gth2[:, t, :],
                    out_offset=None,
                    in_=wword_d.ap(),
                    in_offset=bass.IndirectOffsetOnAxis(
                        ap=ids_sb[:, 2 * P + t : 2 * P + t + 1], axis=0),
                )
            # text = word + pos (in-place, bf16 2x mode)
            vec.tensor_tensor(out=gth2[:], in0=gth2[:], in1=pos01[:], op=Alu.add)

            if any_active:
                for t in range(2):
                    c = 2 * P + t
                    pft = pp_ft.tile([NF, 128], bf, tag="pt")
                    nc.tensor.transpose(out=pft[:], in_=feats[:, c, :],
                                        identity=ident[:])
                    fts = ftspool.tile([NF, 128], bf, tag="fts")
                    vec.tensor_copy(out=fts[:], in_=pft[:])
                    p1 = pp_1.tile([128, PI], f32, tag="p1")
                    nc.tensor.matmul(out=p1[:], lhsT=fts[:], rhs=w1_sb[:],
                                     start=True, stop=True)
                    h = hpool.tile([128, PI], bf, tag="h")
                    nc.scalar.activation(out=h[:], in_=p1[:], func=Act.Gelu,
                                         bias=0.0, scale=1.0)
                    pt0 = pp_t.tile([128, 128], bf, tag="pt")
                    nc.tensor.transpose(out=pt0[:], in_=h[:, 0:128],
                                        identity=ident[:])
                    ht0 = htpool.tile([128, 128], bf, tag="ht0")
                    vec.tensor_copy(out=ht0[:], in_=pt0[:])
                    pt1 = pp_t.tile([128, 128], bf, tag="pt")
                    nc.tensor.transpose(out=pt1[:], in_=h[:, 128:256],
                                        identity=ident[:])
                    ht1 = htpool.tile([128, 128], bf, tag="ht1")
                    vec.tensor_copy(out=ht1[:], in_=pt1[:])
                    py = pp_y.tile([128, H], f32, tag="py")
                    for nb in range(2):
                        sl = slice(nb * 512, (nb + 1) * 512)
                        nc.tensor.matmul(out=py[:, sl], lhsT=ht0[:],
                                         rhs=w2a_sb[:, sl], start=True, stop=False)
                        nc.tensor.matmul(out=py[:, sl], lhsT=ht1[:],
                                         rhs=w2b_sb[:, sl], start=False,
                                         stop=not use_b2)
                        if use_b2:
                            nc.tensor.matmul(out=py[:, sl], lhsT=ones_row[:],
                                             rhs=b2_sb[:, sl], start=False,
                                             stop=True)
                    st2 = smpool.tile([128, 2, 6], f32, tag="st2")
                    vec.bn_stats(out=st2[:, 0, :], in_=py[:, 0:512])
                    vec.bn_stats(out=st2[:, 1, :], in_=py[:, 512:1024])
                    mv2 = smpool.tile([128, 2], f32, tag="mv2")
                    vec.bn_aggr(out=mv2[:], in_=st2[:])
                    sd2 = smpool.tile([128, 1], f32, tag="sd2")
                    nc.scalar.activation(out=sd2[:], in_=mv2[:, 1:2], func=Act.Sqrt,
                                         bias=eps12[:], scale=1.0)
                    r2 = smpool.tile([128, 1], f32, tag="r2")
                    vec.reciprocal(out=r2[:], in_=sd2[:])
                    cm = smpool.tile([128, 1], f32, tag="cm")
                    vec.tensor_tensor(out=cm[:], in0=r2[:], in1=act_f[:, c : c + 1],
                                      op=Alu.mult)
                    dd = smpool.tile([128, 1], f32, tag="dd")
                    vec.tensor_scalar(out=dd[:], in0=mv2[:, 0:1], scalar1=cm[:],
                                      scalar2=-1.0, op0=Alu.mult, op1=Alu.mult)
                    tmp = tpool.tile([128, H], bf, tag="tmp")
                    nc.scalar.activation(out=tmp[:], in_=py[:], func=Act.Identity,
                                         bias=dd[:], scale=cm[:])
                    if use_g2:
                        vec.tensor_tensor(out=tmp[:], in0=tmp[:], in1=g2_sb[:],
                                          op=Alu.mult)
                        mb = tpool.tile([128, H], bf, tag="mb")
                        vec.tensor_scalar(out=mb[:], in0=bg2_sb[:],
                                          scalar1=act_f[:, c : c + 1],
                                          scalar2=None, op0=Alu.mult)
                        vec.tensor_tensor(out=tmp[:], in0=tmp[:], in1=mb[:],
                                          op=Alu.add)
                    vec.tensor_tensor(out=gth2[:, t, :], in0=gth2[:, t, :],
                                      in1=tmp[:], op=Alu.add)

            # ---- final LayerNorm on the pair ----
            stp = smpool.tile([128, 2, 2, 6], f32, tag="stp")
            for t in range(2):
                vec.bn_stats(out=stp[:, t, 0, :], in_=gth2[:, t, 0:512])
                vec.bn_stats(out=stp[:, t, 1, :], in_=gth2[:, t, 512:1024])
            mvp = smpool.tile([128, 2, 2], f32, tag="mvp")
            for t in range(2):
                vec.bn_aggr(out=mvp[:, t, :], in_=stp[:, t, :, :])
            sdp = smpool.tile([128, 2], f32, tag="sdp")
            nc.scalar.activation(out=sdp[:], in_=mvp[:, :, 1], func=Act.Sqrt,
                                 bias=eps12[:], scale=1.0)
            rp = smpool.tile([128, 2], f32, tag="rp")
            vec.reciprocal(out=rp[:], in_=sdp[:])
            nmrp = smpool.tile([128, 2], f32, tag="nmrp")
            vec.tensor_tensor(out=nmrp[:], in0=mvp[:, :, 0], in1=rp[:], op=Alu.mult)
            vec.tensor_scalar(out=nmrp[:], in0=nmrp[:], scalar1=-1.0, scalar2=None,
                              op0=Alu.mult)

            oc2 = opool.tile([128, 2, H], bf, tag="oc")
            for t in range(2):
                nc.scalar.activation(out=oc2[:, t, :], in_=gth2[:, t, :],
                                     func=Act.Identity,
                                     bias=nmrp[:, t : t + 1], scale=rp[:, t : t + 1])
            if use_g1:
                vec.tensor_tensor(out=oc2[:], in0=oc2[:],
                                  in1=_bcast_mid(g1_sb[:]), op=Alu.mult)
                vec.tensor_tensor(out=oc2[:], in0=oc2[:],
                                  in1=_bcast_mid(bg1_sb[:]), op=Alu.add)

            out_ap = out_d.ap()[2 * P : 2 * P + 2].rearrange("c p h -> p c h")
            nc.sync.dma_start(out=out_ap, in_=oc2[:])

    nc.compile()
    return nc


def _bcast_mid(ap):
    """[128, H] -> [128, 2(broadcast), H]"""
    import concourse.bass as bass

    return bass.AP(tensor=ap.tensor, offset=ap.offset,
                   ap=[ap.ap[0], [0, 2], ap.ap[1]])


def _get_nc(flags):
    if flags not in _BUILD_CACHE:
        _BUILD_CACHE[flags] = _build(*flags)
    return _BUILD_CACHE[flags]


def _prep_maps(input_ids, numeric_values, numeric_formats, W_word, W_pos, W_type,
               ln_g, ln_b, p_w1, p_b1, p_w2, p_b2, pln_g, pln_b):
    ids32 = np.ascontiguousarray(input_ids.astype(np.int32))
    fmt32 = np.ascontiguousarray(numeric_formats.astype(np.int32))
    vals = np.ascontiguousarray(numeric_values.astype(np.float32))

    any_active = bool(((ids32 == NUM_TOKEN_ID) & ~np.isnan(vals)).any())
    wword = np.ascontiguousarray(W_word.astype(BF16))
    pos_prime = np.ascontiguousarray((W_pos[:S] + W_type[0]).astype(BF16))  # [S, H]

    w1a = np.zeros((NF, PI), np.float32)
    w1a[:NFEAT] = p_w1
    w1a[NFEAT] = p_b1
    w1a = np.ascontiguousarray(w1a.astype(BF16))
    w2 = np.ascontiguousarray(p_w2.astype(BF16))

    use_b2 = bool(np.any(p_b2 != 0))
    use_g2 = not (np.all(pln_g == 1.0) and np.all(pln_b == 0.0))
    use_g1 = not (np.all(ln_g == 1.0) and np.all(ln_b == 0.0))
    flags = (any_active, use_b2, use_g2, use_g1)

    in_maps = []
    for k in range(NCORES):
        sl = slice(k * SC, (k + 1) * SC)
        # [b, j, p] -> [p, b*2+j]
        ids_t = ids32[:, sl].reshape(B, 2, 128).transpose(2, 0, 1).reshape(128, NT)
        m = {
            "wword": wword,
            "pos": np.ascontiguousarray(
                pos_prime[sl].reshape(2, 128, H).transpose(1, 0, 2)),
            "ids": np.ascontiguousarray(ids_t),
        }
        if any_active:
            vals_t = vals[:, sl].reshape(B, 2, 128).transpose(2, 0, 1).reshape(128, NT)
            fmt_t = fmt32[:, sl].reshape(B, 2, 128).transpose(2, 0, 1).reshape(128, NT)
            m["vals"] = np.ascontiguousarray(vals_t)
            m["fmt"] = np.ascontiguousarray(fmt_t)
            m["w1"] = w1a
            m["w2"] = w2
            if use_b2:
                m["b2"] = np.ascontiguousarray(p_b2[None, :].astype(BF16))
            if use_g2:
                m["g2"] = np.ascontiguousarray(pln_g[None, :].astype(BF16))
                m["bg2"] = np.ascontiguousarray(pln_b[None, :].astype(BF16))
        if use_g1:
            m["g1"] = np.ascontiguousarray(ln_g[None, :].astype(np.float32))
            m["bg1"] = np.ascontiguousarray(ln_b[None, :].astype(np.float32))
        in_maps.append(m)
    return flags, in_maps


def _unshard(results):
    out = np.empty((B, S, H), np.float32)
    for k in range(NCORES):
        r = results[k]["out"].astype(np.float32)  # [NT, 128, H]
        out[:, k * SC : (k + 1) * SC, :] = r.reshape(B, 2, 128, H).reshape(B, SC, H)
    return out


def kernel(**inputs):
    from concourse.bass_utils import run_bass_kernel_spmd

    flags, in_maps = _prep_maps(**inputs)
    nc = _get_nc(flags)
    res = run_bass_kernel_spmd(
        nc, in_maps, core_ids=list(range(NCORES)), trace=TRACE,
    )
    _LAST_RESULT["exec_time_ns"] = res.exec_time_ns
    _LAST_RESULT["mean_exec_time_ns"] = res.mean_exec_time_ns
    _LAST_RESULT["trace"] = res.instructions_and_trace
    return _unshard(res.results)
